# revision 1
# baseline (speedup 1.0000x reference)
"""HANConv Trainium2 kernel (8 NeuronCores, SPMD, full-I/O contract).

Strategy
--------
Destination-sharded, fully core-independent:
  * Each core owns 1/8 of destination nodes for BOTH relations
    (writes: author->paper, written: paper->author).
  * Edges are sorted by destination window (128 dst rows) on host. Per
    window, source rows are gathered from a bf16 copy of the raw source
    features via gpsimd.dma_gather (int16 indices => lo/hi table split),
    and segment-summed with one-hot matmuls accumulating in PSUM.
  * Aggregating RAW features (M = A_norm @ x) lets the relation transform
    and the semantic-score transform both become dense matmuls from M with
    host-folded weights (W_rel, W_rel @ W_sem), so no cross-core exchange
    of transformed features is ever needed.
  * Self path is computed from host-transposed x slices with folded
    weights (W_self, W_self @ W_sem) so no on-chip transpose is needed.
  * 2-candidate semantic softmax == sigmoid of score difference.
"""

import sys

sys.path.insert(0, "/opt/trn_rl_repo")

import numpy as np
import ml_dtypes

import concourse.bacc as bacc
import concourse.mybir as mybir
import concourse.tile as tile
from concourse.bass_utils import run_bass_kernel_spmd

P = 128
N = 50000
D = 256
HALF = 32768  # int16 gather index limit
NCORES = 8
NW_TOTAL = (N + P - 1) // P            # 391 destination windows
NWIN = (NW_TOTAL + NCORES - 1) // NCORES  # 49 windows per core
NW_ALLOC = NWIN * NCORES               # 392 (incl. 1 phantom window)
NPAD = NWIN * P                        # 6272 output rows per core

BF16 = ml_dtypes.bfloat16
F32 = np.float32

# (mps, tps, dps, sb, gbuf, oh) pool bufs
POOL_CFG = (2, 1, 1, 4, 4, 4)


# ---------------------------------------------------------------- host prep
def _prep_relation(row, col):
    """Sort edges by (dst window, src half); pad each group to 128 multiples.

    Returns idx16 [16, NW_ALLOC, 8*call], colf [P, NW_ALLOC, call],
    recip [P, NW_ALLOC], c_lo, c_hi.
    """
    E = row.shape[0]
    key = (col // P) * 2 + (row >= HALF)
    order = np.argsort(key, kind="stable")
    ks = key[order]
    rs = row[order].astype(np.int64)
    cs = col[order].astype(np.int64)

    counts = np.bincount(key, minlength=NW_TOTAL * 2)
    lo_cnt = counts[0::2]
    hi_cnt = counts[1::2]
    c_lo = max(1, int(-(-lo_cnt.max() // P)))
    c_hi = max(1, int(-(-hi_cnt.max() // P)))
    call = c_lo + c_hi

    grp_start = np.zeros(NW_TOTAL * 2 + 1, dtype=np.int64)
    np.cumsum(counts, out=grp_start[1:])
    rank = np.arange(E, dtype=np.int64) - grp_start[ks]
    w_of = ks // 2
    hi_of = ks % 2

    idx_flat = np.zeros(NW_ALLOC * call * P, dtype=np.int16)
    col_flat = np.full(NW_ALLOC * call * P, -1.0, dtype=F32)
    pos = w_of * (call * P) + hi_of * (c_lo * P) + rank
    idx_flat[pos] = (rs - HALF * hi_of).astype(np.int16)
    col_flat[pos] = (cs - w_of * P).astype(F32)

    idx_all = idx_flat.reshape(NW_ALLOC, call * P)
    # wrap for dma_gather: idx i of a gather block -> [i % 16, i // 16]
    lo_wr = idx_all[:, : c_lo * P].reshape(NW_ALLOC, c_lo * 8, 16).transpose(2, 0, 1)
    hi_wr = idx_all[:, c_lo * P:].reshape(NW_ALLOC, c_hi * 8, 16).transpose(2, 0, 1)
    idx16 = np.concatenate([lo_wr, hi_wr], axis=2)  # [16, NW_ALLOC, 8*call]

    colf = col_flat.reshape(NW_ALLOC, call, P).transpose(2, 0, 1)  # [P, NW, call]

    deg = np.bincount(col, minlength=NW_ALLOC * P).astype(F32)[: NW_ALLOC * P]
    recip = (1.0 / np.maximum(deg, 1.0)).reshape(NW_ALLOC, P).T  # [P, NW]
    return idx16, colf, recip, c_lo, c_hi


def _host_prep(inp):
    pr = {}
    pr["wr"] = _prep_relation(np.asarray(inp["row_writes"]), np.asarray(inp["col_writes"]))
    pr["wn"] = _prep_relation(np.asarray(inp["row_written"]), np.asarray(inp["col_written"]))

    xa = np.asarray(inp["x_author"], dtype=F32)
    xp = np.asarray(inp["x_paper"], dtype=F32)
    pr["xba"] = xa.astype(BF16)
    pr["xbp"] = xp.astype(BF16)

    # per-core transposed x slices (for the self path of the dst shard)
    xta, xtp = [], []
    for c in range(NCORES):
        r0, r1 = c * NPAD, min(N, (c + 1) * NPAD)
        sa = np.zeros((D, NPAD), dtype=BF16)
        sp = np.zeros((D, NPAD), dtype=BF16)
        sa[:, : r1 - r0] = xa[r0:r1].T
        sp[:, : r1 - r0] = xp[r0:r1].T
        xta.append(sa)
        xtp.append(sp)
    pr["xta"], pr["xtp"] = xta, xtp

    W_sem = np.asarray(inp["W_sem"], dtype=F32)
    b_sem = np.asarray(inp["b_sem"], dtype=F32)
    w_score = np.asarray(inp["w_score"], dtype=F32)

    def w(name):
        return np.asarray(inp[name], dtype=F32)

    pr["w_self_a"] = w("W_self_author").astype(BF16)
    pr["w_self_p"] = w("W_self_paper").astype(BF16)
    pr["wf_self_a"] = (w("W_self_author") @ W_sem).astype(BF16)
    pr["wf_self_p"] = (w("W_self_paper") @ W_sem).astype(BF16)
    pr["w_rel_wr"] = w("W_rel_writes").astype(BF16)
    pr["w_rel_wn"] = w("W_rel_written").astype(BF16)
    pr["wf_rel_wr"] = (w("W_rel_writes") @ W_sem).astype(BF16)
    pr["wf_rel_wn"] = (w("W_rel_written") @ W_sem).astype(BF16)

    rep = lambda v: np.tile(v.astype(F32), (P, 1))
    pr["b_self_a_rep"] = rep(w("b_self_author"))
    pr["b_self_p_rep"] = rep(w("b_self_paper"))
    pr["bf_self_a_rep"] = rep(w("b_self_author") @ W_sem + b_sem)
    pr["bf_self_p_rep"] = rep(w("b_self_paper") @ W_sem + b_sem)
    pr["bsem_rep"] = rep(b_sem)
    pr["w_rep"] = rep(w_score)

    pr["iota"] = np.tile(np.arange(P, dtype=F32), (P, 1)).astype(BF16)
    pr["ident"] = np.eye(P, dtype=F32).astype(BF16)
    return pr


# ---------------------------------------------------------------- program
def build_program(nwin, c_lo_wr, c_hi_wr, c_lo_wn, c_hi_wn):
    f32 = mybir.dt.float32
    bf16 = mybir.dt.bfloat16
    i16 = mybir.dt.int16
    AF = mybir.ActivationFunctionType
    OP = mybir.AluOpType

    call_wr = c_lo_wr + c_hi_wr
    call_wn = c_lo_wn + c_hi_wn
    npad = nwin * P

    nc = bacc.Bacc("TRN2", target_bir_lowering=False, debug=False)

    _mb, _tb, _db, _sb, _gb, _ob = POOL_CFG

    xba = nc.dram_tensor("xba", [N, D], bf16, kind="ExternalInput")
    xbp = nc.dram_tensor("xbp", [N, D], bf16, kind="ExternalInput")
    xta = nc.dram_tensor("xta", [D, npad], bf16, kind="ExternalInput")
    xtp = nc.dram_tensor("xtp", [D, npad], bf16, kind="ExternalInput")

    wnames = ["w_self_a", "wf_self_a", "w_self_p", "wf_self_p",
              "w_rel_wr", "wf_rel_wr", "w_rel_wn", "wf_rel_wn"]
    wdram = {n: nc.dram_tensor(n, [D, D], bf16, kind="ExternalInput") for n in wnames}
    bnames = ["b_self_a_rep", "bf_self_a_rep", "b_self_p_rep", "bf_self_p_rep",
              "bsem_rep", "w_rep"]
    bdram = {n: nc.dram_tensor(n, [P, D], f32, kind="ExternalInput") for n in bnames}
    iota_d = nc.dram_tensor("iota", [P, P], bf16, kind="ExternalInput")
    ident_d = nc.dram_tensor("ident", [P, P], bf16, kind="ExternalInput")

    idx_wr_d = nc.dram_tensor("idx_wr", [P, nwin * 8 * call_wr], i16, kind="ExternalInput")
    idx_wn_d = nc.dram_tensor("idx_wn", [P, nwin * 8 * call_wn], i16, kind="ExternalInput")
    colf_wr_d = nc.dram_tensor("colf_wr", [P, nwin * call_wr], bf16, kind="ExternalInput")
    colf_wn_d = nc.dram_tensor("colf_wn", [P, nwin * call_wn], bf16, kind="ExternalInput")
    recip_wr_d = nc.dram_tensor("recip_wr", [P, nwin], f32, kind="ExternalInput")
    recip_wn_d = nc.dram_tensor("recip_wn", [P, nwin], f32, kind="ExternalInput")

    oa = nc.dram_tensor("oa", [npad, D], f32, kind="ExternalOutput")
    op_ = nc.dram_tensor("op", [npad, D], f32, kind="ExternalOutput")

    with tile.TileContext(nc) as tc:
        with tc.tile_pool(name="const", bufs=1) as cpool, \
             tc.tile_pool(name="gbuf", bufs=_gb) as gpool, \
             tc.tile_pool(name="oh", bufs=_ob) as ohpool, \
             tc.tile_pool(name="sb", bufs=_sb) as sbpool, \
             tc.tile_pool(name="mps", bufs=_mb, space="PSUM") as mpool, \
             tc.tile_pool(name="tps", bufs=_tb, space="PSUM") as tpool, \
             tc.tile_pool(name="dps", bufs=_db, space="PSUM") as dpool:

            def load(dram, shape, dtype, tag):
                t = cpool.tile(shape, dtype, tag=tag)
                nc.sync.dma_start(t[:], dram)
                return t

            iota_t = load(iota_d[:], [P, P], bf16, "c_iota")
            ident_t = load(ident_d[:], [P, P], bf16, "c_ident")
            wt = {n: (load(wdram[n][0:P, :], [P, D], bf16, f"c_{n}0"),
                      load(wdram[n][P:D, :], [P, D], bf16, f"c_{n}1")) for n in wnames}
            bt = {n: load(bdram[n][:], [P, D], f32, f"c_{n}") for n in bnames}
            xta_t = (load(xta[0:P, :], [P, npad], bf16, "c_xta0"),
                     load(xta[P:D, :], [P, npad], bf16, "c_xta1"))
            xtp_t = (load(xtp[0:P, :], [P, npad], bf16, "c_xtp0"),
                     load(xtp[P:D, :], [P, npad], bf16, "c_xtp1"))
            idx_wr_t = load(idx_wr_d[:], [P, nwin * 8 * call_wr], i16, "c_idxwr")
            idx_wn_t = load(idx_wn_d[:], [P, nwin * 8 * call_wn], i16, "c_idxwn")
            colf_wr_t = load(colf_wr_d[:], [P, nwin * call_wr], bf16, "c_colfwr")
            colf_wn_t = load(colf_wn_d[:], [P, nwin * call_wn], bf16, "c_colfwn")
            recip_wr_t = load(recip_wr_d[:], [P, nwin], f32, "c_recipwr")
            recip_wn_t = load(recip_wn_d[:], [P, nwin], f32, "c_recipwn")

            rels = [
                dict(tag="wr", table=xba, idx=idx_wr_t, colf=colf_wr_t,
                     recip=recip_wr_t, c_lo=c_lo_wr, c_hi=c_hi_wr,
                     xt=xtp_t, w_self=wt["w_self_p"], wf_self=wt["wf_self_p"],
                     w_rel=wt["w_rel_wr"], wf_rel=wt["wf_rel_wr"],
                     b_self=bt["b_self_p_rep"], bf_self=bt["bf_self_p_rep"],
                     out=op_),
                dict(tag="wn", table=xbp, idx=idx_wn_t, colf=colf_wn_t,
                     recip=recip_wn_t, c_lo=c_lo_wn, c_hi=c_hi_wn,
                     xt=xta_t, w_self=wt["w_self_a"], wf_self=wt["wf_self_a"],
                     w_rel=wt["w_rel_wn"], wf_rel=wt["wf_rel_wn"],
                     b_self=bt["b_self_a_rep"], bf_self=bt["bf_self_a_rep"],
                     out=oa),
            ]

            def emit_window(w, r):
                c_lo, c_hi = r["c_lo"], r["c_hi"]
                call = c_lo + c_hi
                ic0 = w * 8 * call

                g_lo = gpool.tile([P, c_lo, D], bf16, tag="glo")
                nc.gpsimd.dma_gather(
                    g_lo[:], r["table"][:], r["idx"][:, ic0: ic0 + 8 * c_lo],
                    c_lo * P, c_lo * P, D, single_packet=False)
                g_hi = gpool.tile([P, c_hi, D], bf16, tag="ghi")
                nc.gpsimd.dma_gather(
                    g_hi[:], r["table"][HALF:, :],
                    r["idx"][:, ic0 + 8 * c_lo: ic0 + 8 * call],
                    c_hi * P, c_hi * P, D, single_packet=False)

                oh = ohpool.tile([P, call, P], bf16, tag="oh")
                nc.vector.tensor_tensor(
                    out=oh[:],
                    in0=r["colf"][:, w * call: (w + 1) * call, None].to_broadcast([P, call, P]),
                    in1=iota_t[:, None, :].to_broadcast([P, call, P]),
                    op=OP.is_equal)

                m_ps = mpool.tile([P, D], f32, tag="m")
                for k in range(call):
                    rhs = g_lo[:, k, :] if k < c_lo else g_hi[:, k - c_lo, :]
                    nc.tensor.matmul(out=m_ps[:], lhsT=oh[:, k, :], rhs=rhs,
                                     start=(k == 0), stop=(k == call - 1))

                m_sb = sbpool.tile([P, D], bf16, tag="m_sb")
                nc.vector.tensor_tensor(
                    out=m_sb[:], in0=m_ps[:],
                    in1=r["recip"][:, w: w + 1].to_broadcast([P, D]), op=OP.mult)

                mt = []
                for h2 in range(2):
                    t_ps = tpool.tile([P, P], bf16, tag="t")
                    nc.tensor.transpose(out=t_ps[:], in_=m_sb[:, h2 * P: (h2 + 1) * P],
                                        identity=ident_t[:])
                    mt_sb = sbpool.tile([P, P], bf16, tag=f"mt{h2}")
                    nc.vector.tensor_copy(out=mt_sb[:], in_=t_ps[:])
                    mt.append(mt_sb)

                def dense(lhsT0, lhsT1, wpair, ptag, pool=dpool):
                    ps = pool.tile([P, D], f32, tag=ptag)
                    nc.tensor.matmul(out=ps[:], lhsT=lhsT0, rhs=wpair[0][:],
                                     start=True, stop=False)
                    nc.tensor.matmul(out=ps[:], lhsT=lhsT1, rhs=wpair[1][:],
                                     start=False, stop=True)
                    return ps

                agg_ps = dense(mt[0][:], mt[1][:], r["w_rel"], "agg")
                sarg_ps = dense(mt[0][:], mt[1][:], r["wf_rel"], "sarg")
                xsl0 = r["xt"][0][:, w * P: (w + 1) * P]
                xsl1 = r["xt"][1][:, w * P: (w + 1) * P]
                h_ps = dense(xsl0, xsl1, r["w_self"], "h")
                sh_ps = dense(xsl0, xsl1, r["wf_self"], "sh")

                def score(ps, brep, stag):
                    targ = sbpool.tile([P, D], f32, tag=f"targ{stag}")
                    nc.vector.tensor_add(out=targ[:], in0=ps[:], in1=brep[:])
                    ttan = sbpool.tile([P, D], f32, tag=f"ttan{stag}")
                    nc.scalar.activation(out=ttan[:], in_=targ[:], func=AF.Tanh)
                    scr = sbpool.tile([P, D], f32, tag=f"scr{stag}")
                    nc.vector.tensor_mul(out=scr[:], in0=ttan[:], in1=bt["w_rep"][:])
                    s = sbpool.tile([P, 1], f32, tag=f"s{stag}")
                    nc.vector.tensor_reduce(out=s[:], in_=scr[:],
                                            axis=mybir.AxisListType.X,
                                            op=OP.add)
                    return s

                s_agg = score(sarg_ps, bt["bsem_rep"], "a")
                s_h = score(sh_ps, r["bf_self"], "h")

                h_sb = sbpool.tile([P, D], f32, tag="h_sb")
                nc.vector.tensor_add(out=h_sb[:], in0=h_ps[:], in1=r["b_self"][:])

                dsc = sbpool.tile([P, 1], f32, tag="dsc")
                nc.vector.tensor_sub(out=dsc[:], in0=s_h[:], in1=s_agg[:])
                a0 = sbpool.tile([P, 1], f32, tag="a0")
                nc.scalar.activation(out=a0[:], in_=dsc[:], func=AF.Sigmoid)

                diff = sbpool.tile([P, D], f32, tag="diff")
                nc.vector.tensor_sub(out=diff[:], in0=h_sb[:], in1=agg_ps[:])
                wd = sbpool.tile([P, D], f32, tag="wd")
                nc.vector.tensor_tensor(out=wd[:], in0=diff[:],
                                        in1=a0[:, 0:1].to_broadcast([P, D]),
                                        op=OP.mult)
                outt = sbpool.tile([P, D], f32, tag="outt")
                nc.vector.tensor_add(out=outt[:], in0=wd[:], in1=agg_ps[:])
                nc.sync.dma_start(r["out"][w * P: (w + 1) * P, :], outt[:])

            for w in range(nwin):
                for r in rels:
                    emit_window(w, r)

    nc.compile()
    return nc


# ---------------------------------------------------------------- driver
_PROG_CACHE = {}


def _get_program(key):
    if key not in _PROG_CACHE:
        _PROG_CACHE[key] = build_program(*key)
    return _PROG_CACHE[key]


def _make_in_maps(pr):
    shared = dict(
        xba=pr["xba"], xbp=pr["xbp"],
        iota=pr["iota"], ident=pr["ident"],
        bsem_rep=pr["bsem_rep"], w_rep=pr["w_rep"],
        b_self_a_rep=pr["b_self_a_rep"], b_self_p_rep=pr["b_self_p_rep"],
        bf_self_a_rep=pr["bf_self_a_rep"], bf_self_p_rep=pr["bf_self_p_rep"],
        w_self_a=pr["w_self_a"], w_self_p=pr["w_self_p"],
        wf_self_a=pr["wf_self_a"], wf_self_p=pr["wf_self_p"],
        w_rel_wr=pr["w_rel_wr"], w_rel_wn=pr["w_rel_wn"],
        wf_rel_wr=pr["wf_rel_wr"], wf_rel_wn=pr["wf_rel_wn"],
    )
    idx_wr, colf_wr, recip_wr, _, _ = pr["wr"]
    idx_wn, colf_wn, recip_wn, _, _ = pr["wn"]
    in_maps = []
    for c in range(NCORES):
        w0, w1 = c * NWIN, (c + 1) * NWIN
        m = dict(shared)
        m["xta"] = pr["xta"][c]
        m["xtp"] = pr["xtp"][c]
        m["idx_wr"] = np.ascontiguousarray(
            np.tile(idx_wr[:, w0:w1].reshape(16, -1), (8, 1)))
        m["idx_wn"] = np.ascontiguousarray(
            np.tile(idx_wn[:, w0:w1].reshape(16, -1), (8, 1)))
        m["colf_wr"] = np.ascontiguousarray(colf_wr[:, w0:w1].reshape(P, -1)).astype(BF16)
        m["colf_wn"] = np.ascontiguousarray(colf_wn[:, w0:w1].reshape(P, -1)).astype(BF16)
        m["recip_wr"] = np.ascontiguousarray(recip_wr[:, w0:w1])
        m["recip_wn"] = np.ascontiguousarray(recip_wn[:, w0:w1])
        in_maps.append(m)
    return in_maps


def run(trace=False, tmpdir=None, **inputs):
    pr = _host_prep(inputs)
    _, _, _, c_lo_wr, c_hi_wr = pr["wr"]
    _, _, _, c_lo_wn, c_hi_wn = pr["wn"]
    nc = _get_program((NWIN, c_lo_wr, c_hi_wr, c_lo_wn, c_hi_wn))
    in_maps = _make_in_maps(pr)
    res = run_bass_kernel_spmd(nc, in_maps, list(range(NCORES)),
                               trace=trace, tmpdir=tmpdir)
    oa = np.empty((N, D), dtype=F32)
    op = np.empty((N, D), dtype=F32)
    for c in range(NCORES):
        r0, r1 = c * NPAD, min(N, (c + 1) * NPAD)
        oa[r0:r1] = res.results[c]["oa"][: r1 - r0]
        op[r0:r1] = res.results[c]["op"][: r1 - r0]
    return (oa, op), res


def kernel(**inputs):
    (oa, op), _ = run(trace=False, **inputs)
    return (oa, op)



# revision 2
# speedup vs baseline: 1.6170x; 1.6170x over previous
"""HANConv Trainium2 kernel (8 NeuronCores, SPMD, full-I/O contract).

Strategy
--------
Destination-sharded, fully core-independent: each core owns 1/8 of the
destination nodes for BOTH relations (writes: author->paper, written:
paper->author).  Edges are sorted by destination window (128 dst rows)
on host; per window the raw source features are gathered from an fp8
(e4m3) copy of the feature tables via gpsimd.dma_gather (int16 indices,
lo/hi table split) and segment-summed with one-hot matmuls into PSUM.
Aggregating RAW features (M = A_norm @ x) lets the relation transform
and the semantic-score transform become dense matmuls from M with
host-folded weights, so no cross-core exchange is ever needed.

Optimizations
-------------
  * fp8 (e4m3) gather tables: half the random-read DMA bytes of bf16;
    output error ~0.9e-2 max-rel, within the 2e-2 budget.
  * fp8 DoubleRow matmuls: the segment-sum consumes two 128-edge chunks
    per PE instruction at double rate.
  * Gathers batched over groups of 4 destination windows (2 calls per
    group vs 8), amortizing the ~2us SWDGE fixed cost per call.
  * Adaptive per-window-slot chunk counts (max over cores, not global
    max): ~8% less gather traffic and segment-sum work.
  * Window-pair batching for the one-hot build, score and blend ops:
    wider DVE/ACT instructions, ~30% fewer instructions overall.
  * Merged dense matmuls: rhs = [W | W @ W_sem] computes the feature
    transform and the semantic-score argument in one PSUM pass.
  * Self path computed from host-transposed bf16 x slices with the same
    folded-weight trick; 2-candidate semantic softmax == sigmoid of the
    score difference.
"""

import sys

sys.path.insert(0, "/opt/trn_rl_repo")

import numpy as np
import ml_dtypes

import concourse.bacc as bacc
import concourse.mybir as mybir
import concourse.tile as tile
from concourse.bass_utils import run_bass_kernel_spmd

P = 128
N = 50000
D = 256
HALF = 32768
NCORES = 8
NW_TOTAL = (N + P - 1) // P            # 391 destination windows
NWIN = (NW_TOTAL + NCORES - 1) // NCORES  # 49 windows per core
NPAD = NWIN * P                        # 6272 output rows per core

BF16 = ml_dtypes.bfloat16
FP8 = ml_dtypes.float8_e4m3fn
F32 = np.float32


# ---------------------------------------------------------------- host prep
def _prep_relation(row, col):
    """Adaptive per-slot packing, pair-grouped for batched gathers.

    Slot j (0..NWIN-1) has chunk counts cl[j], ch[j] = max over cores of
    ceil(count/128).  Per core the flat idx array is ordered per pair t:
      [lo(j0) | lo(j1) | hi(j0) | hi(j1)]  (chunk-padded, idx pad 0)
    and colf is ordered per window: [w: lo chunks then hi chunks], pad -1.

    Returns dict with cl, ch (slot arrays) and per-core idx16 / colf / recip.
    """
    E = row.shape[0]
    row = row.astype(np.int64)
    col = col.astype(np.int64)
    w_of = col // P
    hi_of = (row >= HALF).astype(np.int64)
    key = w_of * 2 + hi_of

    counts = np.bincount(key, minlength=NW_TOTAL * 2)
    # per-core/slot counts; phantom slots (w >= NW_TOTAL) are zero
    cnt = np.zeros((NCORES, NWIN, 2), dtype=np.int64)
    for c in range(NCORES):
        w0 = c * NWIN
        w1 = min(NW_TOTAL, (c + 1) * NWIN)
        cc = counts[w0 * 2: w1 * 2].reshape(-1, 2)
        cnt[c, : w1 - w0] = cc
    cl = np.maximum(1, -(-cnt[:, :, 0].max(axis=0) // P))   # [NWIN]
    ch = np.maximum(1, -(-cnt[:, :, 1].max(axis=0) // P))   # [NWIN]

    GW = 4
    ngrp = (NWIN + GW - 1) // GW

    # block base (in idx slots) for (slot j, half): group-of-GW packed
    # per group g: [lo(j0)..lo(j_{gw-1}) | hi(j0)..hi(j_{gw-1})]
    blk_base = np.zeros((NWIN, 2), dtype=np.int64)
    pos = 0
    for t in range(ngrp):
        js = list(range(GW * t, min(GW * t + GW, NWIN)))
        for j in js:
            blk_base[j, 0] = pos
            pos += cl[j] * P
        for j in js:
            blk_base[j, 1] = pos
            pos += ch[j] * P
    tot_idx = pos

    # colf chunk offsets per window: [w: cl[w] then ch[w]]
    colf_off = np.zeros(NWIN + 1, dtype=np.int64)
    np.cumsum(cl + ch, out=colf_off[1:])
    tot_chunks = int(colf_off[-1])

    # per-edge flat positions
    order = np.argsort(key, kind="stable")
    ks = key[order]
    rs = row[order]
    cs = col[order]
    grp_start = np.zeros(NW_TOTAL * 2 + 1, dtype=np.int64)
    np.cumsum(counts, out=grp_start[1:])
    rank = np.arange(E, dtype=np.int64) - grp_start[ks]
    ew = ks // 2
    eh = ks % 2
    ecore = ew // NWIN
    eslot = ew % NWIN

    idx16 = np.zeros((NCORES, 16, tot_idx // 16), dtype=np.int16)
    colf = np.full((NCORES, P, tot_chunks), -1.0, dtype=np.float32)
    recip = np.zeros((NCORES, P, NWIN), dtype=F32)

    # gather-idx positions (wrapped per 16)
    gpos = blk_base[eslot, eh] + rank
    idx_val = (rs - HALF * eh).astype(np.int16)
    idx16[ecore, gpos % 16, gpos // 16] = idx_val

    # colf positions: [P, chunks]: edge at slot-local position rank ->
    # chunk = rank // P, partition = rank % P
    ch_idx = colf_off[eslot] + eh * cl[eslot] + rank // P
    cpart = rank % P
    colf[ecore, cpart, ch_idx] = (cs - ew * P).astype(np.float32)

    deg = np.bincount(col, minlength=NCORES * NPAD).astype(F32)[: NCORES * NPAD]
    recip[:] = (1.0 / np.maximum(deg, 1.0)).reshape(NCORES, NWIN, P).transpose(0, 2, 1)

    return dict(cl=cl, ch=ch, idx16=idx16, colf=colf, recip=recip,
                tot_idx=tot_idx, tot_chunks=tot_chunks)


def _host_prep(inp):
    pr = {}
    pr["wr"] = _prep_relation(np.asarray(inp["row_writes"]),
                              np.asarray(inp["col_writes"]))
    pr["wn"] = _prep_relation(np.asarray(inp["row_written"]),
                              np.asarray(inp["col_written"]))

    xa = np.asarray(inp["x_author"], dtype=F32)
    xp = np.asarray(inp["x_paper"], dtype=F32)
    pr["xba"] = xa.astype(FP8)
    pr["xbp"] = xp.astype(FP8)

    xta, xtp = [], []
    for c in range(NCORES):
        r0, r1 = c * NPAD, min(N, (c + 1) * NPAD)
        sa = np.zeros((D, NPAD), dtype=BF16)
        sp = np.zeros((D, NPAD), dtype=BF16)
        sa[:, : r1 - r0] = xa[r0:r1].T
        sp[:, : r1 - r0] = xp[r0:r1].T
        xta.append(sa)
        xtp.append(sp)
    pr["xta"], pr["xtp"] = xta, xtp

    W_sem = np.asarray(inp["W_sem"], dtype=F32)
    b_sem = np.asarray(inp["b_sem"], dtype=F32)
    w_score = np.asarray(inp["w_score"], dtype=F32)

    def w(name):
        return np.asarray(inp[name], dtype=F32)

    # merged [W | W @ W_sem] weights
    pr["wc_rel_wr"] = np.concatenate(
        [w("W_rel_writes"), w("W_rel_writes") @ W_sem], axis=1).astype(BF16)
    pr["wc_rel_wn"] = np.concatenate(
        [w("W_rel_written"), w("W_rel_written") @ W_sem], axis=1).astype(BF16)
    pr["wc_self_a"] = np.concatenate(
        [w("W_self_author"), w("W_self_author") @ W_sem], axis=1).astype(BF16)
    pr["wc_self_p"] = np.concatenate(
        [w("W_self_paper"), w("W_self_paper") @ W_sem], axis=1).astype(BF16)

    rep = lambda v: np.tile(v.astype(F32), (P, 1))
    # self bias cat: [b_self | b_self@W_sem + b_sem]
    pr["bc_self_a"] = rep(np.concatenate(
        [w("b_self_author"), w("b_self_author") @ W_sem + b_sem]))
    pr["bc_self_p"] = rep(np.concatenate(
        [w("b_self_paper"), w("b_self_paper") @ W_sem + b_sem]))
    pr["bsem_rep"] = rep(b_sem)
    pr["w_rep"] = rep(w_score)

    pr["iota"] = np.tile(np.arange(P, dtype=F32), (P, 1)).astype(BF16)
    pr["ident"] = np.eye(P, dtype=F32).astype(BF16)
    return pr


# ---------------------------------------------------------------- program
def build_program(cl_wr, ch_wr, cl_wn, ch_wn,
                  ti_wr, tc_wr, ti_wn, tc_wn,
                  gcfg=(2, 3), dcfg=(1, 2, 1), scfg=(3, 2)):
    """cl/ch: per-slot chunk-count tuples. ti/tc: tot_idx, tot_chunks."""
    f32 = mybir.dt.float32
    bf16 = mybir.dt.bfloat16
    fp8 = mybir.dt.float8e4
    i16 = mybir.dt.int16
    AF = mybir.ActivationFunctionType
    OP = mybir.AluOpType
    DR = mybir.MatmulPerfMode.DoubleRow

    gbufs, ohbufs = gcfg
    mbufs, tbufs, dbufs = dcfg
    sbufs, s2bufs = scfg

    GW = 4
    ngrp = (NWIN + GW - 1) // GW
    npair = (NWIN + 1) // 2

    nc = bacc.Bacc("TRN2", target_bir_lowering=False, debug=False)

    xba = nc.dram_tensor("xba", [N, D], fp8, kind="ExternalInput")
    xbp = nc.dram_tensor("xbp", [N, D], fp8, kind="ExternalInput")
    xta = nc.dram_tensor("xta", [D, NPAD], bf16, kind="ExternalInput")
    xtp = nc.dram_tensor("xtp", [D, NPAD], bf16, kind="ExternalInput")

    wnames = ["wc_rel_wr", "wc_rel_wn", "wc_self_a", "wc_self_p"]
    wdram = {n: nc.dram_tensor(n, [D, 2 * D], bf16, kind="ExternalInput")
             for n in wnames}
    bnames = ["bc_self_a", "bc_self_p"]
    bdram = {n: nc.dram_tensor(n, [P, 2 * D], f32, kind="ExternalInput")
             for n in bnames}
    bsem_d = nc.dram_tensor("bsem_rep", [P, D], f32, kind="ExternalInput")
    wrep_d = nc.dram_tensor("w_rep", [P, D], f32, kind="ExternalInput")
    iota_d = nc.dram_tensor("iota", [P, P], bf16, kind="ExternalInput")
    ident_d = nc.dram_tensor("ident", [P, P], bf16, kind="ExternalInput")

    idx_wr_d = nc.dram_tensor("idx_wr", [P, ti_wr // 16], i16, kind="ExternalInput")
    idx_wn_d = nc.dram_tensor("idx_wn", [P, ti_wn // 16], i16, kind="ExternalInput")
    colf_wr_d = nc.dram_tensor("colf_wr", [P, tc_wr], bf16, kind="ExternalInput")
    colf_wn_d = nc.dram_tensor("colf_wn", [P, tc_wn], bf16, kind="ExternalInput")
    recip_wr_d = nc.dram_tensor("recip_wr", [P, NWIN], f32, kind="ExternalInput")
    recip_wn_d = nc.dram_tensor("recip_wn", [P, NWIN], f32, kind="ExternalInput")

    oa = nc.dram_tensor("oa", [NPAD, D], f32, kind="ExternalOutput")
    op_ = nc.dram_tensor("op", [NPAD, D], f32, kind="ExternalOutput")

    # max group chunk counts for gather tile sizing; max pair chunk count
    # for the one-hot tile
    def group_sizes(cl, ch):
        mlo = mhi = 0
        for t in range(ngrp):
            js = list(range(GW * t, min(GW * t + GW, NWIN)))
            mlo = max(mlo, sum(cl[j] for j in js))
            mhi = max(mhi, sum(ch[j] for j in js))
        return mlo, mhi

    def pair_tot(cl, ch):
        m = 0
        for t in range(npair):
            j0, j1 = 2 * t, 2 * t + 1
            m = max(m, cl[j0] + ch[j0]
                    + ((cl[j1] + ch[j1]) if j1 < NWIN else 0))
        return m

    mlo_wr, mhi_wr = group_sizes(cl_wr, ch_wr)
    mlo_wn, mhi_wn = group_sizes(cl_wn, ch_wn)
    mlo = max(mlo_wr, mlo_wn)
    mhi = max(mhi_wr, mhi_wn)
    mtot = max(pair_tot(cl_wr, ch_wr), pair_tot(cl_wn, ch_wn))

    with tile.TileContext(nc) as tc_:
        with tc_.tile_pool(name="const", bufs=1) as cpool, \
             tc_.tile_pool(name="gbuf", bufs=gbufs) as gpool, \
             tc_.tile_pool(name="oh", bufs=ohbufs) as ohpool, \
             tc_.tile_pool(name="sb", bufs=sbufs) as sbpool, \
             tc_.tile_pool(name="sb2", bufs=s2bufs) as s2pool, \
             tc_.tile_pool(name="mps", bufs=mbufs, space="PSUM") as mpool, \
             tc_.tile_pool(name="tps", bufs=tbufs, space="PSUM") as tpool, \
             tc_.tile_pool(name="dps", bufs=dbufs, space="PSUM") as dpool:

            def load(dram, shape, dtype, tag):
                t = cpool.tile(shape, dtype, tag=tag)
                nc.sync.dma_start(t[:], dram)
                return t

            iota_t = load(iota_d[:], [P, P], bf16, "c_iota")
            ident_t = load(ident_d[:], [P, P], bf16, "c_ident")
            wt = {n: (load(wdram[n][0:P, :], [P, 2 * D], bf16, f"c_{n}0"),
                      load(wdram[n][P:D, :], [P, 2 * D], bf16, f"c_{n}1"))
                  for n in wnames}
            bct = {n: load(bdram[n][:], [P, 2 * D], f32, f"c_{n}") for n in bnames}
            bsem_t = load(bsem_d[:], [P, D], f32, "c_bsem")
            wrep_t = load(wrep_d[:], [P, D], f32, "c_wrep")
            xta_t = (load(xta[0:P, :], [P, NPAD], bf16, "c_xta0"),
                     load(xta[P:D, :], [P, NPAD], bf16, "c_xta1"))
            xtp_t = (load(xtp[0:P, :], [P, NPAD], bf16, "c_xtp0"),
                     load(xtp[P:D, :], [P, NPAD], bf16, "c_xtp1"))
            idx_wr_t = load(idx_wr_d[:], [P, ti_wr // 16], i16, "c_idxwr")
            idx_wn_t = load(idx_wn_d[:], [P, ti_wn // 16], i16, "c_idxwn")
            colf_wr_t = load(colf_wr_d[:], [P, tc_wr], bf16, "c_colfwr")
            colf_wn_t = load(colf_wn_d[:], [P, tc_wn], bf16, "c_colfwn")
            recip_wr_t = load(recip_wr_d[:], [P, NWIN], f32, "c_recipwr")
            recip_wn_t = load(recip_wn_d[:], [P, NWIN], f32, "c_recipwn")

            rels = [
                dict(tag="wr", table=xba, idx=idx_wr_t, colf=colf_wr_t,
                     recip=recip_wr_t, cl=cl_wr, ch=ch_wr,
                     xt=xtp_t, wc_self=wt["wc_self_p"], wc_rel=wt["wc_rel_wr"],
                     bc_self=bct["bc_self_p"], out=op_),
                dict(tag="wn", table=xbp, idx=idx_wn_t, colf=colf_wn_t,
                     recip=recip_wn_t, cl=cl_wn, ch=ch_wn,
                     xt=xta_t, wc_self=wt["wc_self_a"], wc_rel=wt["wc_rel_wn"],
                     bc_self=bct["bc_self_a"], out=oa),
            ]
            # python-side offsets per relation (group-of-GW layout)
            for r in rels:
                cl, ch = r["cl"], r["ch"]
                off_lo, off_hi = [], []
                glo_b, ghi_b = {}, {}
                pos = 0
                for t in range(ngrp):
                    js = list(range(GW * t, min(GW * t + GW, NWIN)))
                    off_lo.append(pos // 16)
                    b = 0
                    for j in js:
                        glo_b[j] = b
                        b += cl[j]
                        pos += cl[j] * P
                    off_hi.append(pos // 16)
                    b = 0
                    for j in js:
                        ghi_b[j] = b
                        b += ch[j]
                        pos += ch[j] * P
                cfs = []
                cf = 0
                for j in range(NWIN):
                    cfs.append(cf)
                    cf += cl[j] + ch[j]
                r["off_lo"], r["off_hi"] = off_lo, off_hi
                r["glo_b"], r["ghi_b"], r["cfs"] = glo_b, ghi_b, cfs

            def emit_group(t, r):
                cl, ch = r["cl"], r["ch"]
                js = list(range(GW * t, min(GW * t + GW, NWIN)))
                nlo = sum(cl[j] for j in js)
                nhi = sum(ch[j] for j in js)

                g_lo = gpool.tile([P, mlo, D], fp8, tag="glo")
                nc.gpsimd.dma_gather(
                    g_lo[:, 0:nlo, :], r["table"][:],
                    r["idx"][:, r["off_lo"][t]: r["off_lo"][t] + nlo * 8],
                    nlo * P, nlo * P, D, single_packet=False)
                g_hi = gpool.tile([P, mhi, D], fp8, tag="ghi")
                nc.gpsimd.dma_gather(
                    g_hi[:, 0:nhi, :], r["table"][HALF:, :],
                    r["idx"][:, r["off_hi"][t]: r["off_hi"][t] + nhi * 8],
                    nhi * P, nhi * P, D, single_packet=False)
                return g_lo, g_hi

            def emit_pair(pt, r, g_lo, g_hi):
                cl, ch = r["cl"], r["ch"]
                j0, j1 = 2 * pt, 2 * pt + 1
                two = j1 < NWIN
                cl0, ch0 = cl[j0], ch[j0]
                cl1 = cl[j1] if two else 0
                ch1 = ch[j1] if two else 0
                ncw = cl0 + cl1 + ch0 + ch1
                nb = 2 if two else 1

                cf0 = r["cfs"][j0]
                oh = ohpool.tile([P, mtot, P], fp8, tag="oh")
                nc.vector.tensor_tensor(
                    out=oh[:, 0:ncw, :],
                    in0=r["colf"][:, cf0: cf0 + ncw, None].to_broadcast([P, ncw, P]),
                    in1=iota_t[:, None, :].to_broadcast([P, ncw, P]),
                    op=OP.is_equal)

                # segment-sum into per-window PSUM tiles.  fp8 DoubleRow
                # consumes two chunks per matmul at 2x rate; odd leftovers
                # use a regular fp8 matmul.
                m_sb = sbpool.tile([P, 2, D], bf16, tag="m_sb")
                for j, (wj, clw, chw) in enumerate(
                        [(j0, cl0, ch0), (j1, cl1, ch1)][:nb]):
                    gb_lo = r["glo_b"][wj]
                    gb_hi = r["ghi_b"][wj]
                    ohb = (cl0 + ch0) * j
                    m_ps = mpool.tile([P, D], f32, tag=f"m{j}")
                    steps = []
                    k = 0
                    while k < clw:   # lo chunks
                        if k + 1 < clw:
                            steps.append((gb_lo + k, ohb + k, 2, g_lo))
                            k += 2
                        else:
                            steps.append((gb_lo + k, ohb + k, 1, g_lo))
                            k += 1
                    k = 0
                    while k < chw:   # hi chunks
                        if k + 1 < chw:
                            steps.append((gb_hi + k, ohb + clw + k, 2, g_hi))
                            k += 2
                        else:
                            steps.append((gb_hi + k, ohb + clw + k, 1, g_hi))
                            k += 1
                    for si, (gb, ob, nk, gt) in enumerate(steps):
                        st = si == 0
                        sp = si == len(steps) - 1
                        if nk == 2:
                            nc.tensor.matmul(
                                out=m_ps[:], lhsT=oh[:, ob: ob + 2, :],
                                rhs=gt[:, gb: gb + 2, :], start=st, stop=sp,
                                perf_mode=DR)
                        else:
                            nc.tensor.matmul(
                                out=m_ps[:], lhsT=oh[:, ob, :],
                                rhs=gt[:, gb, :], start=st, stop=sp)
                    nc.vector.tensor_tensor(
                        out=m_sb[:, j, :], in0=m_ps[:],
                        in1=r["recip"][:, j0 + j: j0 + j + 1].to_broadcast([P, D]),
                        op=OP.mult)

                mts = []
                for j in range(nb):
                    for h2 in range(2):
                        t_ps = tpool.tile([P, P], bf16, tag="t")
                        nc.tensor.transpose(
                            out=t_ps[:], in_=m_sb[:, j, h2 * P: (h2 + 1) * P],
                            identity=ident_t[:])
                        mt_sb = sbpool.tile([P, P], bf16, tag=f"mt{j}{h2}")
                        nc.vector.tensor_copy(out=mt_sb[:], in_=t_ps[:])
                        mts.append(mt_sb)

                # dense: agg_cat = m @ [W_rel | W_rel@W_sem]
                agg_ps = dpool.tile([P, 2, 2 * D], f32, tag="aggc")
                self_ps = dpool.tile([P, 2, 2 * D], f32, tag="selfc")
                for j in range(nb):
                    nc.tensor.matmul(out=agg_ps[:, j, :], lhsT=mts[2 * j][:],
                                     rhs=r["wc_rel"][0][:], start=True, stop=False)
                    nc.tensor.matmul(out=agg_ps[:, j, :], lhsT=mts[2 * j + 1][:],
                                     rhs=r["wc_rel"][1][:], start=False, stop=True)
                    xsl0 = r["xt"][0][:, (j0 + j) * P: (j0 + j + 1) * P]
                    xsl1 = r["xt"][1][:, (j0 + j) * P: (j0 + j + 1) * P]
                    nc.tensor.matmul(out=self_ps[:, j, :], lhsT=xsl0,
                                     rhs=r["wc_self"][0][:], start=True, stop=False)
                    nc.tensor.matmul(out=self_ps[:, j, :], lhsT=xsl1,
                                     rhs=r["wc_self"][1][:], start=False, stop=True)

                # score/blend (batched over the pair); PSUM evacuated early so
                # dps bufs=1 does not stall the next pair's dense matmuls
                selfc = s2pool.tile([P, 2, 2 * D], f32, tag="selfc_sb")
                nc.vector.tensor_add(
                    out=selfc[:, 0:nb, :], in0=self_ps[:, 0:nb, :],
                    in1=r["bc_self"][:, None, :].to_broadcast([P, nb, 2 * D]))
                targ_a = sbpool.tile([P, 2, D], f32, tag="targ_a")
                nc.vector.tensor_add(
                    out=targ_a[:, 0:nb, :], in0=agg_ps[:, 0:nb, D: 2 * D],
                    in1=bsem_t[:, None, :].to_broadcast([P, nb, D]))
                agg_sb = sbpool.tile([P, 2, D], f32, tag="agg_sb")
                nc.vector.tensor_copy(out=agg_sb[:, 0:nb, :],
                                      in_=agg_ps[:, 0:nb, 0:D])

                # in-place chains: targ_a -> tanh -> *w -> s_a
                nc.scalar.activation(out=targ_a[:, 0:nb, :], in_=targ_a[:, 0:nb, :],
                                     func=AF.Tanh)
                nc.scalar.activation(out=selfc[:, 0:nb, D: 2 * D],
                                     in_=selfc[:, 0:nb, D: 2 * D], func=AF.Tanh)
                nc.vector.tensor_tensor(
                    out=targ_a[:, 0:nb, :], in0=targ_a[:, 0:nb, :],
                    in1=wrep_t[:, None, :].to_broadcast([P, nb, D]), op=OP.mult)
                nc.vector.tensor_tensor(
                    out=selfc[:, 0:nb, D: 2 * D], in0=selfc[:, 0:nb, D: 2 * D],
                    in1=wrep_t[:, None, :].to_broadcast([P, nb, D]), op=OP.mult)

                s_a = sbpool.tile([P, 2, 1], f32, tag="s_a")
                nc.vector.tensor_reduce(out=s_a[:, 0:nb, :], in_=targ_a[:, 0:nb, :],
                                        axis=mybir.AxisListType.X, op=OP.add)
                s_h = sbpool.tile([P, 2, 1], f32, tag="s_h")
                nc.vector.tensor_reduce(out=s_h[:, 0:nb, :],
                                        in_=selfc[:, 0:nb, D: 2 * D],
                                        axis=mybir.AxisListType.X, op=OP.add)

                dsc = sbpool.tile([P, 2, 1], f32, tag="dsc")
                nc.vector.tensor_sub(out=dsc[:, 0:nb, :], in0=s_h[:, 0:nb, :],
                                     in1=s_a[:, 0:nb, :])
                a0 = sbpool.tile([P, 2, 1], f32, tag="a0")
                nc.scalar.activation(out=a0[:, 0:nb, :], in_=dsc[:, 0:nb, :],
                                     func=AF.Sigmoid)

                # diff -> *a0 -> +agg (in place)
                diff = sbpool.tile([P, 2, D], f32, tag="diff")
                nc.vector.tensor_sub(out=diff[:, 0:nb, :],
                                     in0=selfc[:, 0:nb, 0:D],
                                     in1=agg_sb[:, 0:nb, :])
                nc.vector.tensor_tensor(
                    out=diff[:, 0:nb, :], in0=diff[:, 0:nb, :],
                    in1=a0[:, 0:nb, 0:1].to_broadcast([P, nb, D]), op=OP.mult)
                nc.vector.tensor_add(out=diff[:, 0:nb, :], in0=diff[:, 0:nb, :],
                                     in1=agg_sb[:, 0:nb, :])
                for j in range(nb):
                    nc.sync.dma_start(r["out"][(j0 + j) * P: (j0 + j + 1) * P, :],
                                      diff[:, j, :])

            for t in range(ngrp):
                for r in rels:
                    g_lo, g_hi = emit_group(t, r)
                    for pt in range(2 * t, min(2 * t + 2, npair)):
                        emit_pair(pt, r, g_lo, g_hi)

    nc.compile()
    return nc


# ---------------------------------------------------------------- driver
_PROG_CACHE = {}


def _get_program(key):
    if key not in _PROG_CACHE:
        cl_wr, ch_wr, cl_wn, ch_wn, ti_wr, tc_wr, ti_wn, tc_wn = key
        _PROG_CACHE[key] = build_program(
            np.array(cl_wr), np.array(ch_wr), np.array(cl_wn), np.array(ch_wn),
            ti_wr, tc_wr, ti_wn, tc_wn)
    return _PROG_CACHE[key]


def _prog_key(pr):
    return (tuple(pr["wr"]["cl"]), tuple(pr["wr"]["ch"]),
            tuple(pr["wn"]["cl"]), tuple(pr["wn"]["ch"]),
            pr["wr"]["tot_idx"], pr["wr"]["tot_chunks"],
            pr["wn"]["tot_idx"], pr["wn"]["tot_chunks"])


def _make_in_maps(pr):
    shared = dict(
        xba=pr["xba"], xbp=pr["xbp"],
        iota=pr["iota"], ident=pr["ident"],
        bsem_rep=pr["bsem_rep"], w_rep=pr["w_rep"],
        bc_self_a=pr["bc_self_a"], bc_self_p=pr["bc_self_p"],
        wc_rel_wr=pr["wc_rel_wr"], wc_rel_wn=pr["wc_rel_wn"],
        wc_self_a=pr["wc_self_a"], wc_self_p=pr["wc_self_p"],
    )
    in_maps = []
    for c in range(NCORES):
        m = dict(shared)
        m["xta"] = pr["xta"][c]
        m["xtp"] = pr["xtp"][c]
        m["idx_wr"] = np.ascontiguousarray(
            np.tile(pr["wr"]["idx16"][c], (8, 1)))
        m["idx_wn"] = np.ascontiguousarray(
            np.tile(pr["wn"]["idx16"][c], (8, 1)))
        m["colf_wr"] = pr["wr"]["colf"][c].astype(BF16)
        m["colf_wn"] = pr["wn"]["colf"][c].astype(BF16)
        m["recip_wr"] = np.ascontiguousarray(pr["wr"]["recip"][c])
        m["recip_wn"] = np.ascontiguousarray(pr["wn"]["recip"][c])
        in_maps.append(m)
    return in_maps


def run(trace=False, tmpdir=None, **inputs):
    pr = _host_prep(inputs)
    nc = _get_program(_prog_key(pr))
    in_maps = _make_in_maps(pr)
    res = run_bass_kernel_spmd(nc, in_maps, list(range(NCORES)),
                               trace=trace, tmpdir=tmpdir)
    oa = np.empty((N, D), dtype=F32)
    op = np.empty((N, D), dtype=F32)
    for c in range(NCORES):
        r0, r1 = c * NPAD, min(N, (c + 1) * NPAD)
        oa[r0:r1] = res.results[c]["oa"][: r1 - r0]
        op[r0:r1] = res.results[c]["op"][: r1 - r0]
    return (oa, op), res


def kernel(**inputs):
    (oa, op), _ = run(trace=False, **inputs)
    return (oa, op)


# revision 4
# speedup vs baseline: 1.8602x; 1.1504x over previous
"""HANConv Trainium2 kernel (8 NeuronCores, SPMD, full-I/O contract).

Strategy
--------
Destination-sharded, fully core-independent: each core owns 1/8 of the
destination nodes for BOTH relations (writes: author->paper, written:
paper->author).  Edges are sorted by destination window (128 dst rows)
on host; per window the raw source features are gathered from an fp8
(e4m3) copy of the feature tables via gpsimd.dma_gather (int16 indices,
lo/hi table split) and segment-summed with one-hot matmuls into PSUM.
Aggregating RAW features (M = A_norm @ x) lets the relation transform
and the semantic-score transform become dense matmuls from M with
host-folded weights, so no cross-core exchange is ever needed.

Optimizations
-------------
  * fp8 (e4m3) gather tables: half the random-read DMA bytes of bf16;
    output error ~0.9e-2 max-rel, within the 2e-2 budget.
  * fp8 DoubleRow matmuls: the segment-sum consumes two 128-edge chunks
    per PE instruction at double rate.
  * Gathers batched over groups of 4 destination windows (2 calls per
    group vs 8), amortizing the ~2us SWDGE fixed cost per call.
  * Adaptive per-window-slot chunk counts (max over cores, not global
    max): ~8% less gather traffic and segment-sum work.
  * Window-pair batching for the one-hot build, score and blend ops:
    wider DVE/ACT instructions, ~30% fewer instructions overall.
  * Merged dense matmuls: rhs = [W | W @ W_sem] computes the feature
    transform and the semantic-score argument in one PSUM pass.
  * Self path computed from host-transposed bf16 x slices with the same
    folded-weight trick; 2-candidate semantic softmax == sigmoid of the
    score difference.
  * Engine balancing: PSUM evacuations (recip scaling, transpose copies)
    run on the ACT engine (activation Copy, per-partition scale AP) and
    the tanh/score chain runs in bf16, relieving the vector engine.
"""

import sys

sys.path.insert(0, "/opt/trn_rl_repo")

import numpy as np
import ml_dtypes

import concourse.bacc as bacc
import concourse.mybir as mybir
import concourse.tile as tile
from concourse.bass_utils import run_bass_kernel_spmd

P = 128
N = 50000
D = 256
HALF = 32768
NCORES = 8
NW_TOTAL = (N + P - 1) // P            # 391 destination windows
NWIN = (NW_TOTAL + NCORES - 1) // NCORES  # 49 windows per core
NPAD = NWIN * P                        # 6272 output rows per core

BF16 = ml_dtypes.bfloat16
FP8 = ml_dtypes.float8_e4m3fn
F32 = np.float32


# ---------------------------------------------------------------- host prep
def _prep_relation(row, col):
    """Adaptive per-slot packing, pair-grouped for batched gathers.

    Slot j (0..NWIN-1) has chunk counts cl[j], ch[j] = max over cores of
    ceil(count/128).  Per core the flat idx array is ordered per pair t:
      [lo(j0) | lo(j1) | hi(j0) | hi(j1)]  (chunk-padded, idx pad 0)
    and colf is ordered per window: [w: lo chunks then hi chunks], pad -1.

    Returns dict with cl, ch (slot arrays) and per-core idx16 / colf / recip.
    """
    E = row.shape[0]
    row = row.astype(np.int64)
    col = col.astype(np.int64)
    w_of = col // P
    hi_of = (row >= HALF).astype(np.int64)
    key = w_of * 2 + hi_of

    counts = np.bincount(key, minlength=NW_TOTAL * 2)
    # per-core/slot counts; phantom slots (w >= NW_TOTAL) are zero
    cnt = np.zeros((NCORES, NWIN, 2), dtype=np.int64)
    for c in range(NCORES):
        w0 = c * NWIN
        w1 = min(NW_TOTAL, (c + 1) * NWIN)
        cc = counts[w0 * 2: w1 * 2].reshape(-1, 2)
        cnt[c, : w1 - w0] = cc
    cl = np.maximum(1, -(-cnt[:, :, 0].max(axis=0) // P))   # [NWIN]
    ch = np.maximum(1, -(-cnt[:, :, 1].max(axis=0) // P))   # [NWIN]

    GW = 4
    ngrp = (NWIN + GW - 1) // GW

    # block base (in idx slots) for (slot j, half): group-of-GW packed
    # per group g: [lo(j0)..lo(j_{gw-1}) | hi(j0)..hi(j_{gw-1})]
    blk_base = np.zeros((NWIN, 2), dtype=np.int64)
    pos = 0
    for t in range(ngrp):
        js = list(range(GW * t, min(GW * t + GW, NWIN)))
        for j in js:
            blk_base[j, 0] = pos
            pos += cl[j] * P
        for j in js:
            blk_base[j, 1] = pos
            pos += ch[j] * P
    tot_idx = pos

    # colf chunk offsets per window: [w: cl[w] then ch[w]]
    colf_off = np.zeros(NWIN + 1, dtype=np.int64)
    np.cumsum(cl + ch, out=colf_off[1:])
    tot_chunks = int(colf_off[-1])

    # per-edge flat positions
    order = np.argsort(key, kind="stable")
    ks = key[order]
    rs = row[order]
    cs = col[order]
    grp_start = np.zeros(NW_TOTAL * 2 + 1, dtype=np.int64)
    np.cumsum(counts, out=grp_start[1:])
    rank = np.arange(E, dtype=np.int64) - grp_start[ks]
    ew = ks // 2
    eh = ks % 2
    ecore = ew // NWIN
    eslot = ew % NWIN

    idx16 = np.zeros((NCORES, 16, tot_idx // 16), dtype=np.int16)
    colf = np.full((NCORES, P, tot_chunks), -1.0, dtype=np.float32)
    recip = np.zeros((NCORES, P, NWIN), dtype=F32)

    # gather-idx positions (wrapped per 16)
    gpos = blk_base[eslot, eh] + rank
    idx_val = (rs - HALF * eh).astype(np.int16)
    idx16[ecore, gpos % 16, gpos // 16] = idx_val

    # colf positions: [P, chunks]: edge at slot-local position rank ->
    # chunk = rank // P, partition = rank % P
    ch_idx = colf_off[eslot] + eh * cl[eslot] + rank // P
    cpart = rank % P
    colf[ecore, cpart, ch_idx] = (cs - ew * P).astype(np.float32)

    deg = np.bincount(col, minlength=NCORES * NPAD).astype(F32)[: NCORES * NPAD]
    recip[:] = (1.0 / np.maximum(deg, 1.0)).reshape(NCORES, NWIN, P).transpose(0, 2, 1)

    return dict(cl=cl, ch=ch, idx16=idx16, colf=colf, recip=recip,
                tot_idx=tot_idx, tot_chunks=tot_chunks)


def _host_prep(inp):
    pr = {}
    pr["wr"] = _prep_relation(np.asarray(inp["row_writes"]),
                              np.asarray(inp["col_writes"]))
    pr["wn"] = _prep_relation(np.asarray(inp["row_written"]),
                              np.asarray(inp["col_written"]))

    xa = np.asarray(inp["x_author"], dtype=F32)
    xp = np.asarray(inp["x_paper"], dtype=F32)
    pr["xba"] = xa.astype(FP8)
    pr["xbp"] = xp.astype(FP8)

    xta, xtp = [], []
    for c in range(NCORES):
        r0, r1 = c * NPAD, min(N, (c + 1) * NPAD)
        sa = np.zeros((D, NPAD), dtype=BF16)
        sp = np.zeros((D, NPAD), dtype=BF16)
        sa[:, : r1 - r0] = xa[r0:r1].T
        sp[:, : r1 - r0] = xp[r0:r1].T
        xta.append(sa)
        xtp.append(sp)
    pr["xta"], pr["xtp"] = xta, xtp

    W_sem = np.asarray(inp["W_sem"], dtype=F32)
    b_sem = np.asarray(inp["b_sem"], dtype=F32)
    w_score = np.asarray(inp["w_score"], dtype=F32)

    def w(name):
        return np.asarray(inp[name], dtype=F32)

    # merged [W | W @ W_sem] weights
    pr["wc_rel_wr"] = np.concatenate(
        [w("W_rel_writes"), w("W_rel_writes") @ W_sem], axis=1).astype(BF16)
    pr["wc_rel_wn"] = np.concatenate(
        [w("W_rel_written"), w("W_rel_written") @ W_sem], axis=1).astype(BF16)
    pr["wc_self_a"] = np.concatenate(
        [w("W_self_author"), w("W_self_author") @ W_sem], axis=1).astype(BF16)
    pr["wc_self_p"] = np.concatenate(
        [w("W_self_paper"), w("W_self_paper") @ W_sem], axis=1).astype(BF16)

    rep = lambda v: np.tile(v.astype(F32), (P, 1))
    # self bias cat: [b_self | b_self@W_sem + b_sem]
    pr["bc_self_a"] = rep(np.concatenate(
        [w("b_self_author"), w("b_self_author") @ W_sem + b_sem]))
    pr["bc_self_p"] = rep(np.concatenate(
        [w("b_self_paper"), w("b_self_paper") @ W_sem + b_sem]))
    pr["bsem_rep"] = rep(b_sem)
    pr["w_rep"] = rep(w_score)

    pr["iota"] = np.tile(np.arange(P, dtype=F32), (P, 1)).astype(BF16)
    pr["ident"] = np.eye(P, dtype=F32).astype(BF16)
    return pr


# ---------------------------------------------------------------- program
def build_program(cl_wr, ch_wr, cl_wn, ch_wn,
                  ti_wr, tc_wr, ti_wn, tc_wn,
                  gcfg=(2, 3), dcfg=(1, 2, 1), scfg=(3, 2)):
    """cl/ch: per-slot chunk-count tuples. ti/tc: tot_idx, tot_chunks."""
    f32 = mybir.dt.float32
    bf16 = mybir.dt.bfloat16
    fp8 = mybir.dt.float8e4
    i16 = mybir.dt.int16
    AF = mybir.ActivationFunctionType
    OP = mybir.AluOpType
    DR = mybir.MatmulPerfMode.DoubleRow

    gbufs, ohbufs = gcfg
    mbufs, tbufs, dbufs = dcfg
    sbufs, s2bufs = scfg

    GW = 4
    ngrp = (NWIN + GW - 1) // GW
    npair = (NWIN + 1) // 2

    nc = bacc.Bacc("TRN2", target_bir_lowering=False, debug=False)

    xba = nc.dram_tensor("xba", [N, D], fp8, kind="ExternalInput")
    xbp = nc.dram_tensor("xbp", [N, D], fp8, kind="ExternalInput")
    xta = nc.dram_tensor("xta", [D, NPAD], bf16, kind="ExternalInput")
    xtp = nc.dram_tensor("xtp", [D, NPAD], bf16, kind="ExternalInput")

    wnames = ["wc_rel_wr", "wc_rel_wn", "wc_self_a", "wc_self_p"]
    wdram = {n: nc.dram_tensor(n, [D, 2 * D], bf16, kind="ExternalInput")
             for n in wnames}
    bnames = ["bc_self_a", "bc_self_p"]
    bdram = {n: nc.dram_tensor(n, [P, 2 * D], f32, kind="ExternalInput")
             for n in bnames}
    bsem_d = nc.dram_tensor("bsem_rep", [P, D], f32, kind="ExternalInput")
    wrep_d = nc.dram_tensor("w_rep", [P, D], f32, kind="ExternalInput")
    iota_d = nc.dram_tensor("iota", [P, P], bf16, kind="ExternalInput")
    ident_d = nc.dram_tensor("ident", [P, P], bf16, kind="ExternalInput")

    idx_wr_d = nc.dram_tensor("idx_wr", [P, ti_wr // 16], i16, kind="ExternalInput")
    idx_wn_d = nc.dram_tensor("idx_wn", [P, ti_wn // 16], i16, kind="ExternalInput")
    colf_wr_d = nc.dram_tensor("colf_wr", [P, tc_wr], bf16, kind="ExternalInput")
    colf_wn_d = nc.dram_tensor("colf_wn", [P, tc_wn], bf16, kind="ExternalInput")
    recip_wr_d = nc.dram_tensor("recip_wr", [P, NWIN], f32, kind="ExternalInput")
    recip_wn_d = nc.dram_tensor("recip_wn", [P, NWIN], f32, kind="ExternalInput")

    oa = nc.dram_tensor("oa", [NPAD, D], f32, kind="ExternalOutput")
    op_ = nc.dram_tensor("op", [NPAD, D], f32, kind="ExternalOutput")

    # max group chunk counts for gather tile sizing; max pair chunk count
    # for the one-hot tile
    def group_sizes(cl, ch):
        mlo = mhi = 0
        for t in range(ngrp):
            js = list(range(GW * t, min(GW * t + GW, NWIN)))
            mlo = max(mlo, sum(cl[j] for j in js))
            mhi = max(mhi, sum(ch[j] for j in js))
        return mlo, mhi

    def pair_tot(cl, ch):
        m = 0
        for t in range(npair):
            j0, j1 = 2 * t, 2 * t + 1
            m = max(m, cl[j0] + ch[j0]
                    + ((cl[j1] + ch[j1]) if j1 < NWIN else 0))
        return m

    mlo_wr, mhi_wr = group_sizes(cl_wr, ch_wr)
    mlo_wn, mhi_wn = group_sizes(cl_wn, ch_wn)
    mlo = max(mlo_wr, mlo_wn)
    mhi = max(mhi_wr, mhi_wn)
    mtot = max(pair_tot(cl_wr, ch_wr), pair_tot(cl_wn, ch_wn))

    with tile.TileContext(nc) as tc_:
        with tc_.tile_pool(name="const", bufs=1) as cpool, \
             tc_.tile_pool(name="gbuf", bufs=gbufs) as gpool, \
             tc_.tile_pool(name="oh", bufs=ohbufs) as ohpool, \
             tc_.tile_pool(name="sb", bufs=sbufs) as sbpool, \
             tc_.tile_pool(name="sb2", bufs=s2bufs) as s2pool, \
             tc_.tile_pool(name="mps", bufs=mbufs, space="PSUM") as mpool, \
             tc_.tile_pool(name="tps", bufs=tbufs, space="PSUM") as tpool, \
             tc_.tile_pool(name="dps", bufs=dbufs, space="PSUM") as dpool:

            def load(dram, shape, dtype, tag):
                t = cpool.tile(shape, dtype, tag=tag)
                nc.sync.dma_start(t[:], dram)
                return t

            iota_t = load(iota_d[:], [P, P], bf16, "c_iota")
            ident_t = load(ident_d[:], [P, P], bf16, "c_ident")
            wt = {n: (load(wdram[n][0:P, :], [P, 2 * D], bf16, f"c_{n}0"),
                      load(wdram[n][P:D, :], [P, 2 * D], bf16, f"c_{n}1"))
                  for n in wnames}
            bct = {n: load(bdram[n][:], [P, 2 * D], f32, f"c_{n}") for n in bnames}
            bsem_t = load(bsem_d[:], [P, D], f32, "c_bsem")
            wrep_t = load(wrep_d[:], [P, D], f32, "c_wrep")
            xta_t = (load(xta[0:P, :], [P, NPAD], bf16, "c_xta0"),
                     load(xta[P:D, :], [P, NPAD], bf16, "c_xta1"))
            xtp_t = (load(xtp[0:P, :], [P, NPAD], bf16, "c_xtp0"),
                     load(xtp[P:D, :], [P, NPAD], bf16, "c_xtp1"))
            idx_wr_t = load(idx_wr_d[:], [P, ti_wr // 16], i16, "c_idxwr")
            idx_wn_t = load(idx_wn_d[:], [P, ti_wn // 16], i16, "c_idxwn")
            colf_wr_t = load(colf_wr_d[:], [P, tc_wr], bf16, "c_colfwr")
            colf_wn_t = load(colf_wn_d[:], [P, tc_wn], bf16, "c_colfwn")
            recip_wr_t = load(recip_wr_d[:], [P, NWIN], f32, "c_recipwr")
            recip_wn_t = load(recip_wn_d[:], [P, NWIN], f32, "c_recipwn")

            rels = [
                dict(tag="wr", table=xba, idx=idx_wr_t, colf=colf_wr_t,
                     recip=recip_wr_t, cl=cl_wr, ch=ch_wr,
                     xt=xtp_t, wc_self=wt["wc_self_p"], wc_rel=wt["wc_rel_wr"],
                     bc_self=bct["bc_self_p"], out=op_),
                dict(tag="wn", table=xbp, idx=idx_wn_t, colf=colf_wn_t,
                     recip=recip_wn_t, cl=cl_wn, ch=ch_wn,
                     xt=xta_t, wc_self=wt["wc_self_a"], wc_rel=wt["wc_rel_wn"],
                     bc_self=bct["bc_self_a"], out=oa),
            ]
            # python-side offsets per relation (group-of-GW layout)
            for r in rels:
                cl, ch = r["cl"], r["ch"]
                off_lo, off_hi = [], []
                glo_b, ghi_b = {}, {}
                pos = 0
                for t in range(ngrp):
                    js = list(range(GW * t, min(GW * t + GW, NWIN)))
                    off_lo.append(pos // 16)
                    b = 0
                    for j in js:
                        glo_b[j] = b
                        b += cl[j]
                        pos += cl[j] * P
                    off_hi.append(pos // 16)
                    b = 0
                    for j in js:
                        ghi_b[j] = b
                        b += ch[j]
                        pos += ch[j] * P
                cfs = []
                cf = 0
                for j in range(NWIN):
                    cfs.append(cf)
                    cf += cl[j] + ch[j]
                r["off_lo"], r["off_hi"] = off_lo, off_hi
                r["glo_b"], r["ghi_b"], r["cfs"] = glo_b, ghi_b, cfs

            def emit_group(t, r):
                cl, ch = r["cl"], r["ch"]
                js = list(range(GW * t, min(GW * t + GW, NWIN)))
                nlo = sum(cl[j] for j in js)
                nhi = sum(ch[j] for j in js)

                g_lo = gpool.tile([P, mlo, D], fp8, tag="glo")
                nc.gpsimd.dma_gather(
                    g_lo[:, 0:nlo, :], r["table"][:],
                    r["idx"][:, r["off_lo"][t]: r["off_lo"][t] + nlo * 8],
                    nlo * P, nlo * P, D, single_packet=False)
                g_hi = gpool.tile([P, mhi, D], fp8, tag="ghi")
                nc.gpsimd.dma_gather(
                    g_hi[:, 0:nhi, :], r["table"][HALF:, :],
                    r["idx"][:, r["off_hi"][t]: r["off_hi"][t] + nhi * 8],
                    nhi * P, nhi * P, D, single_packet=False)
                return g_lo, g_hi

            def emit_pair(pt, r, g_lo, g_hi):
                cl, ch = r["cl"], r["ch"]
                j0, j1 = 2 * pt, 2 * pt + 1
                two = j1 < NWIN
                cl0, ch0 = cl[j0], ch[j0]
                cl1 = cl[j1] if two else 0
                ch1 = ch[j1] if two else 0
                ncw = cl0 + cl1 + ch0 + ch1
                nb = 2 if two else 1

                cf0 = r["cfs"][j0]
                oh = ohpool.tile([P, mtot, P], fp8, tag="oh")
                nc.vector.tensor_tensor(
                    out=oh[:, 0:ncw, :],
                    in0=r["colf"][:, cf0: cf0 + ncw, None].to_broadcast([P, ncw, P]),
                    in1=iota_t[:, None, :].to_broadcast([P, ncw, P]),
                    op=OP.is_equal)

                # segment-sum into per-window PSUM tiles.  fp8 DoubleRow
                # consumes two chunks per matmul at 2x rate; odd leftovers
                # use a regular fp8 matmul.
                m_sb = sbpool.tile([P, 2, D], bf16, tag="m_sb")
                for j, (wj, clw, chw) in enumerate(
                        [(j0, cl0, ch0), (j1, cl1, ch1)][:nb]):
                    gb_lo = r["glo_b"][wj]
                    gb_hi = r["ghi_b"][wj]
                    ohb = (cl0 + ch0) * j
                    m_ps = mpool.tile([P, D], f32, tag=f"m{j}")
                    steps = []
                    k = 0
                    while k < clw:   # lo chunks
                        if k + 1 < clw:
                            steps.append((gb_lo + k, ohb + k, 2, g_lo))
                            k += 2
                        else:
                            steps.append((gb_lo + k, ohb + k, 1, g_lo))
                            k += 1
                    k = 0
                    while k < chw:   # hi chunks
                        if k + 1 < chw:
                            steps.append((gb_hi + k, ohb + clw + k, 2, g_hi))
                            k += 2
                        else:
                            steps.append((gb_hi + k, ohb + clw + k, 1, g_hi))
                            k += 1
                    for si, (gb, ob, nk, gt) in enumerate(steps):
                        st = si == 0
                        sp = si == len(steps) - 1
                        if nk == 2:
                            nc.tensor.matmul(
                                out=m_ps[:], lhsT=oh[:, ob: ob + 2, :],
                                rhs=gt[:, gb: gb + 2, :], start=st, stop=sp,
                                perf_mode=DR)
                        else:
                            nc.tensor.matmul(
                                out=m_ps[:], lhsT=oh[:, ob, :],
                                rhs=gt[:, gb, :], start=st, stop=sp)
                    nc.scalar.activation(
                        out=m_sb[:, j, :], in_=m_ps[:], func=AF.Copy,
                        scale=r["recip"][:, j0 + j: j0 + j + 1])

                mts = []
                for j in range(nb):
                    for h2 in range(2):
                        t_ps = tpool.tile([P, P], bf16, tag="t")
                        nc.tensor.transpose(
                            out=t_ps[:], in_=m_sb[:, j, h2 * P: (h2 + 1) * P],
                            identity=ident_t[:])
                        mt_sb = sbpool.tile([P, P], bf16, tag=f"mt{j}{h2}")
                        nc.scalar.activation(out=mt_sb[:], in_=t_ps[:],
                                             func=AF.Copy)
                        mts.append(mt_sb)

                # dense: agg_cat = m @ [W_rel | W_rel@W_sem]
                agg_ps = dpool.tile([P, 2, 2 * D], f32, tag="aggc")
                self_ps = dpool.tile([P, 2, 2 * D], f32, tag="selfc")
                for j in range(nb):
                    nc.tensor.matmul(out=agg_ps[:, j, :], lhsT=mts[2 * j][:],
                                     rhs=r["wc_rel"][0][:], start=True, stop=False)
                    nc.tensor.matmul(out=agg_ps[:, j, :], lhsT=mts[2 * j + 1][:],
                                     rhs=r["wc_rel"][1][:], start=False, stop=True)
                    xsl0 = r["xt"][0][:, (j0 + j) * P: (j0 + j + 1) * P]
                    xsl1 = r["xt"][1][:, (j0 + j) * P: (j0 + j + 1) * P]
                    nc.tensor.matmul(out=self_ps[:, j, :], lhsT=xsl0,
                                     rhs=r["wc_self"][0][:], start=True, stop=False)
                    nc.tensor.matmul(out=self_ps[:, j, :], lhsT=xsl1,
                                     rhs=r["wc_self"][1][:], start=False, stop=True)

                # score/blend (batched over the pair); PSUM evacuated early so
                # dps bufs=1 does not stall the next pair's dense matmuls
                h_sb = s2pool.tile([P, 2, D], f32, tag="h_sb")
                nc.vector.tensor_add(
                    out=h_sb[:, 0:nb, :], in0=self_ps[:, 0:nb, 0:D],
                    in1=r["bc_self"][:, None, 0:D].to_broadcast([P, nb, D]))
                targ_h = sbpool.tile([P, 2, D], bf16, tag="targ_h")
                nc.vector.tensor_add(
                    out=targ_h[:, 0:nb, :], in0=self_ps[:, 0:nb, D: 2 * D],
                    in1=r["bc_self"][:, None, D: 2 * D].to_broadcast([P, nb, D]))
                targ_a = sbpool.tile([P, 2, D], bf16, tag="targ_a")
                nc.vector.tensor_add(
                    out=targ_a[:, 0:nb, :], in0=agg_ps[:, 0:nb, D: 2 * D],
                    in1=bsem_t[:, None, :].to_broadcast([P, nb, D]))
                agg_sb = sbpool.tile([P, 2, D], f32, tag="agg_sb")
                nc.vector.tensor_copy(out=agg_sb[:, 0:nb, :],
                                      in_=agg_ps[:, 0:nb, 0:D])

                # in-place bf16 chains: targ -> tanh -> *w -> reduce
                nc.scalar.activation(out=targ_a[:, 0:nb, :], in_=targ_a[:, 0:nb, :],
                                     func=AF.Tanh)
                nc.scalar.activation(out=targ_h[:, 0:nb, :], in_=targ_h[:, 0:nb, :],
                                     func=AF.Tanh)
                nc.vector.tensor_tensor(
                    out=targ_a[:, 0:nb, :], in0=targ_a[:, 0:nb, :],
                    in1=wrep_t[:, None, :].to_broadcast([P, nb, D]), op=OP.mult)
                nc.vector.tensor_tensor(
                    out=targ_h[:, 0:nb, :], in0=targ_h[:, 0:nb, :],
                    in1=wrep_t[:, None, :].to_broadcast([P, nb, D]), op=OP.mult)

                s_a = sbpool.tile([P, 2, 1], f32, tag="s_a")
                nc.vector.tensor_reduce(out=s_a[:, 0:nb, :], in_=targ_a[:, 0:nb, :],
                                        axis=mybir.AxisListType.X, op=OP.add)
                s_h = sbpool.tile([P, 2, 1], f32, tag="s_h")
                nc.vector.tensor_reduce(out=s_h[:, 0:nb, :],
                                        in_=targ_h[:, 0:nb, :],
                                        axis=mybir.AxisListType.X, op=OP.add)

                dsc = sbpool.tile([P, 2, 1], f32, tag="dsc")
                nc.vector.tensor_sub(out=dsc[:, 0:nb, :], in0=s_h[:, 0:nb, :],
                                     in1=s_a[:, 0:nb, :])
                a0 = sbpool.tile([P, 2, 1], f32, tag="a0")
                nc.scalar.activation(out=a0[:, 0:nb, :], in_=dsc[:, 0:nb, :],
                                     func=AF.Sigmoid)

                # diff -> *a0 -> +agg (in place)
                diff = sbpool.tile([P, 2, D], f32, tag="diff")
                nc.vector.tensor_sub(out=diff[:, 0:nb, :],
                                     in0=h_sb[:, 0:nb, :],
                                     in1=agg_sb[:, 0:nb, :])
                nc.vector.tensor_tensor(
                    out=diff[:, 0:nb, :], in0=diff[:, 0:nb, :],
                    in1=a0[:, 0:nb, 0:1].to_broadcast([P, nb, D]), op=OP.mult)
                nc.vector.tensor_add(out=diff[:, 0:nb, :], in0=diff[:, 0:nb, :],
                                     in1=agg_sb[:, 0:nb, :])
                for j in range(nb):
                    nc.sync.dma_start(r["out"][(j0 + j) * P: (j0 + j + 1) * P, :],
                                      diff[:, j, :])

            for t in range(ngrp):
                for r in rels:
                    g_lo, g_hi = emit_group(t, r)
                    for pt in range(2 * t, min(2 * t + 2, npair)):
                        emit_pair(pt, r, g_lo, g_hi)

    nc.compile()
    return nc


# ---------------------------------------------------------------- driver
_PROG_CACHE = {}


def _get_program(key):
    if key not in _PROG_CACHE:
        cl_wr, ch_wr, cl_wn, ch_wn, ti_wr, tc_wr, ti_wn, tc_wn = key
        _PROG_CACHE[key] = build_program(
            np.array(cl_wr), np.array(ch_wr), np.array(cl_wn), np.array(ch_wn),
            ti_wr, tc_wr, ti_wn, tc_wn)
    return _PROG_CACHE[key]


def _prog_key(pr):
    return (tuple(pr["wr"]["cl"]), tuple(pr["wr"]["ch"]),
            tuple(pr["wn"]["cl"]), tuple(pr["wn"]["ch"]),
            pr["wr"]["tot_idx"], pr["wr"]["tot_chunks"],
            pr["wn"]["tot_idx"], pr["wn"]["tot_chunks"])


def _make_in_maps(pr):
    shared = dict(
        xba=pr["xba"], xbp=pr["xbp"],
        iota=pr["iota"], ident=pr["ident"],
        bsem_rep=pr["bsem_rep"], w_rep=pr["w_rep"],
        bc_self_a=pr["bc_self_a"], bc_self_p=pr["bc_self_p"],
        wc_rel_wr=pr["wc_rel_wr"], wc_rel_wn=pr["wc_rel_wn"],
        wc_self_a=pr["wc_self_a"], wc_self_p=pr["wc_self_p"],
    )
    in_maps = []
    for c in range(NCORES):
        m = dict(shared)
        m["xta"] = pr["xta"][c]
        m["xtp"] = pr["xtp"][c]
        m["idx_wr"] = np.ascontiguousarray(
            np.tile(pr["wr"]["idx16"][c], (8, 1)))
        m["idx_wn"] = np.ascontiguousarray(
            np.tile(pr["wn"]["idx16"][c], (8, 1)))
        m["colf_wr"] = pr["wr"]["colf"][c].astype(BF16)
        m["colf_wn"] = pr["wn"]["colf"][c].astype(BF16)
        m["recip_wr"] = np.ascontiguousarray(pr["wr"]["recip"][c])
        m["recip_wn"] = np.ascontiguousarray(pr["wn"]["recip"][c])
        in_maps.append(m)
    return in_maps


def run(trace=False, tmpdir=None, **inputs):
    pr = _host_prep(inputs)
    nc = _get_program(_prog_key(pr))
    in_maps = _make_in_maps(pr)
    res = run_bass_kernel_spmd(nc, in_maps, list(range(NCORES)),
                               trace=trace, tmpdir=tmpdir)
    oa = np.empty((N, D), dtype=F32)
    op = np.empty((N, D), dtype=F32)
    for c in range(NCORES):
        r0, r1 = c * NPAD, min(N, (c + 1) * NPAD)
        oa[r0:r1] = res.results[c]["oa"][: r1 - r0]
        op[r0:r1] = res.results[c]["op"][: r1 - r0]
    return (oa, op), res


def kernel(**inputs):
    (oa, op), _ = run(trace=False, **inputs)
    return (oa, op)
